# revision 1
# baseline (speedup 1.0000x reference)
# Depthwise causal conv2d (N=2, C=16, H=W=2048, kernel 6x11) on 8 TRN2 cores.
#
# y[b,c,p,q] = sum_{r,s} w[c,r,s] * xm[b,c, p+r-5, q+s-5], xm = tril-masked x,
# y tril-masked.  Sharding: the 32 (b,c) images are independent; 4 per core.
#
# Per-core compute: for each output tile of [M<=123 rows, Nd<=512 cols], the
# row-tap contraction is a banded-Toeplitz matmul: for each of the S=11
# column taps, out[m, n] += band_s[k, m] * x[k, n+s] where band_s[k, m] =
# w[c, k-m, s] (k-m in [0,6)).  11 accumulating matmuls per tile into one
# PSUM bank.  Tiles fully above the causal diagonal are never computed or
# written (output DRAM is pre-zeroed); tiles crossing it get the causal mask
# applied to the input (gpsimd affine_select in SBUF) and to the output
# (DVE multiply with a staircase 0/1 tile during PSUM evacuation).
import sys

sys.path.insert(0, "/opt/trn_rl_repo")

import numpy as np

import concourse.bacc as bacc
import concourse.mybir as mybir
import concourse.tile as tile
from concourse.bass_utils import run_bass_kernel_spmd

N, C, H, W = 2, 16, 2048, 2048
R, S, PH, PW = 6, 11, 5, 5
NCORES = 8
IPC = (N * C) // NCORES  # images per core
MT = 123  # output rows per row-tile (128 input rows incl. 5-row halo)
NTS = 512  # max output cols per tile (one PSUM bank of fp32)
BANDW = 128  # allocated band width (cols used: M)
STAIR_W = 1536  # staircase mask width
STAIR_C = 511  # staircase offset: stair[i, u] = 1 iff u <= i + STAIR_C
F32 = mybir.dt.float32

# Matmul input dtype. Measured per-core kernel time / rel err vs fp32 ref:
#   "bf16" 392 us / 2.3e-3,  "f16" 474 us / 3.1e-4,  "f32r" 568 us / 2.5e-4.
DTYPE_MODE = "bf16"

_NC_CACHE = {}


def _xdt():
    return {
        "f16": mybir.dt.float16,
        "bf16": mybir.dt.bfloat16,
        "f32r": mybir.dt.float32r,
    }[DTYPE_MODE]


def _np_xdt():
    if DTYPE_MODE == "f16":
        return np.dtype(np.float16)
    if DTYPE_MODE == "bf16":
        import ml_dtypes

        return np.dtype(ml_dtypes.bfloat16)
    return np.dtype(np.float32)


def _row_tiles():
    out = []
    p0 = 0
    while p0 < H:
        out.append((p0, min(MT, H - p0)))
        p0 += MT
    return out


def _col_tiles(pmax):
    """Column tiles covering q in [0, pmax]; width 512 except the last,
    which shrinks to a multiple of 128 (fp32r needs >=256 for full rate)."""
    min_nd = 256 if DTYPE_MODE == "f32r" else 128
    out = []
    q0 = 0
    while q0 <= pmax:
        needed = pmax - q0 + 1
        if needed >= NTS:
            nd = NTS
        else:
            nd = min(NTS, max(min_nd, 128 * ((needed + 127) // 128)))
        out.append((q0, nd))
        q0 += nd
    return out


def _build_program(rep=1):
    """One SPMD program: conv of IPC images [H, W] with per-image bands.

    rep > 1 wraps the whole body in a hardware loop executing it `rep`
    times — benchmarking only (amplifies kernel time above the fixed
    dispatch overhead of the execution path)."""
    import contextlib

    xdt = _xdt()
    nc = bacc.Bacc("TRN2", target_bir_lowering=False, debug=False,
                   num_devices=NCORES)
    x = nc.dram_tensor("x", [IPC, H, W], xdt, kind="ExternalInput")
    bands = nc.dram_tensor("bands", [IPC, 128, S * BANDW], xdt,
                           kind="ExternalInput")
    y = nc.dram_tensor("y", [IPC, H, W], F32, kind="ExternalOutput")

    row_tiles = _row_tiles()

    with tile.TileContext(nc) as tc:
        with (
            tc.tile_pool(name="const", bufs=1) as cpool,
            tc.tile_pool(name="xin", bufs=6) as xpool,
            tc.tile_pool(name="out", bufs=4) as opool,
            tc.tile_pool(name="psum", bufs=8, space="PSUM") as ppool,
            tc.For_i(0, rep, 1) if rep > 1 else contextlib.nullcontext(),
        ):
            # Per-image Toeplitz bands, resident for the whole kernel.
            bt = cpool.tile([128, IPC * S * BANDW], xdt)
            for i in range(IPC):
                nc.sync.dma_start(
                    out=bt[:, i * S * BANDW:(i + 1) * S * BANDW],
                    in_=bands[i],
                )
            # Staircase causal mask: stair[i, u] = 1 iff i + STAIR_C - u >= 0.
            stair = cpool.tile([128, STAIR_W], F32)
            nc.gpsimd.memset(stair[:], 1.0)
            nc.gpsimd.affine_select(
                out=stair[:], in_=stair[:],
                compare_op=mybir.AluOpType.is_ge, fill=0.0,
                base=STAIR_C, channel_multiplier=1,
                pattern=[[-1, STAIR_W]],
            )

            for i in range(IPC):
                band_i = bt[:, i * S * BANDW:(i + 1) * S * BANDW]
                for (p0, M) in row_tiles:
                    pmax = p0 + M - 1
                    for (q0, nd) in _col_tiles(pmax):
                        _emit_tile(nc, tc, xpool, opool, ppool, x, y, band_i,
                                   stair, i, p0, M, pmax, q0, nd)
    nc.compile()
    return nc


def _emit_tile(nc, tc, xpool, opool, ppool, x, y, band_i, stair,
               i, p0, M, pmax, q0, nd):
    xdt = _xdt()
    inw = nd + S - 1  # input tile width incl. halo
    # Input rows [p0-PH, p0+M), cols [q0-PW, q0+nd+PW) clipped to the image
    # and (on the right) to the causal extent pmax.
    h0 = p0 - PH
    hv0, hv1 = max(0, h0), min(H, p0 + M)
    w0 = q0 - PW
    wv0 = max(0, w0)
    wv1 = min(W, q0 + nd + PW, pmax + 1)

    d = p0 - q0  # diagonal offset of this tile
    # Causal mask on input needed iff the loaded region touches h < w.
    in_mask = h0 < wv1 - 1
    # Causal mask on output needed iff the tile crosses p < q.
    out_mask = p0 < q0 + nd - 1

    xt = xpool.tile([128, inw], xdt, tag="xin")
    nc.sync.dma_start(
        out=xt[hv0 - h0:hv1 - h0, wv0 - w0:wv1 - w0],
        in_=x[i, hv0:hv1, wv0:wv1],
    )
    if q0 == 0:
        # w in [-5, 0) is zero padding that the causal select keeps
        # (memset can't write fp32r): keep iff j >= PW.
        nc.gpsimd.affine_select(
            out=xt[:], in_=xt[:],
            compare_op=mybir.AluOpType.is_ge, fill=0.0,
            base=-PW, channel_multiplier=0,
            pattern=[[1, inw]],
        )
    if in_mask:
        # keep iff (h0 + k) >= (w0 + j)  <=>  k - j + (h0 - w0) >= 0.
        # Also zeroes the unloaded right-of-pmax region and, for p0 == 0,
        # the k < PH halo rows above the image (h < 0 keeps nothing).
        nc.gpsimd.affine_select(
            out=xt[:], in_=xt[:],
            compare_op=mybir.AluOpType.is_ge, fill=0.0,
            base=h0 - w0, channel_multiplier=1,
            pattern=[[-1, inw]],
        )

    pt = ppool.tile([M, NTS], F32, tag="psum")
    for s in range(S):
        nc.tensor.matmul(
            pt[:, :nd],
            lhsT=band_i[:, s * BANDW:s * BANDW + M],
            rhs=xt[:, s:s + nd],
            start=(s == 0), stop=(s == S - 1),
        )

    # Columns q > pmax are entirely above the diagonal: skip them.
    wn = min(nd, pmax - q0 + 1)
    ot = opool.tile([128, NTS], F32, tag="out")
    if out_mask:
        # Evacuate PSUM through the causal staircase: keep iff
        # (p0+m) >= (q0+n) <=> stair[m, n + STAIR_C - d] with d = p0-q0.
        u0 = STAIR_C - d
        nc.vector.tensor_mul(
            ot[:M, :wn], pt[:M, :wn], stair[:M, u0:u0 + wn],
        )
    else:
        nc.any.tensor_copy(ot[:M, :wn], pt[:M, :wn])
    nc.sync.dma_start(
        out=y[i, p0:p0 + M, q0:q0 + wn],
        in_=ot[:M, :wn],
    )


def _build_bands(weight):
    """Host-side: per-image banded Toeplitz weights.
    bands[img, k, s*BANDW + m] = w[c(img), k-m, s] for k-m in [0, R)."""
    nimg = N * C
    bands = np.zeros((nimg, 128, S * BANDW), np.float32)
    m = np.arange(BANDW)
    for s in range(S):
        for r in range(R):
            # band[m+r, s*BANDW+m] = w[c, r, s]
            valid = m + r < 128
            mv = m[valid]
            for img in range(nimg):
                c = img % C
                bands[img, mv + r, s * BANDW + mv] = weight[c, r, s]
    return bands.astype(_np_xdt())


def kernel(x, weight):
    x = np.asarray(x, dtype=np.float32)
    weight = np.asarray(weight, dtype=np.float32)
    assert x.shape == (N, C, H, W) and weight.shape == (C, R, S)

    if "nc" not in _NC_CACHE:
        _NC_CACHE["nc"] = _build_program()
    nc = _NC_CACHE["nc"]

    x_imgs = np.ascontiguousarray(x.reshape(N * C, H, W)).astype(
        _np_xdt(), copy=False)
    bands = _build_bands(weight)
    in_maps = [
        {
            "x": x_imgs[k * IPC:(k + 1) * IPC],
            "bands": bands[k * IPC:(k + 1) * IPC],
        }
        for k in range(NCORES)
    ]
    res = run_bass_kernel_spmd(nc, in_maps, list(range(NCORES)))
    out = np.concatenate([res.results[k]["y"] for k in range(NCORES)], axis=0)
    return out.reshape(N, C, H, W)



# revision 3
# speedup vs baseline: 1.7743x; 1.7743x over previous
# Depthwise causal conv2d (N=2, C=16, H=W=2048, kernel 6x11) on 8 TRN2 cores.
#
# v2: s-outer matmul schedule. Per (image, row-tile of 123 output rows):
#   - ONE x DMA loads the full causal extent [128 rows, pmax+11 cols] (bf16),
#     one gpsimd causal affine_select on the ~133 diagonal columns, one memset
#     of the 5-col left pad.
#   - Loop s = 0..10 OUTER, column-tiles INNER: the band for tap s is the
#     stationary operand for up to 4 consecutive matmuls (one per column
#     tile), amortizing LDWEIGHTS 4x and letting the PE reorder window hide
#     the next load behind the current matmuls.
#   - Column tiles are exact-width (no 128-rounding waste).
#   - Evacuation (DVE staircase mask / copy) writes bf16 into one whole-row
#     SBUF tile; ONE y DMA per row-tile.  y is bf16 in DRAM (host converts
#     to fp32); upper triangle comes from the donated zero output buffer.
import sys

sys.path.insert(0, "/opt/trn_rl_repo")

import numpy as np

import concourse.bacc as bacc
import concourse.mybir as mybir
import concourse.tile as tile
from concourse.bass_utils import run_bass_kernel_spmd

N, C, H, W = 2, 16, 2048, 2048
R, S, PH, PW = 6, 11, 5, 5
NCORES = 8
IPC = (N * C) // NCORES  # images per core
MT = 123  # output rows per row-tile (128 input rows incl. 5-row halo)
NTS = 512  # max output cols per tile (one PSUM bank of fp32)
BANDW = 128  # allocated band width (cols used: M)
STAIR_W = 1536  # staircase mask width
STAIR_C = 511  # staircase offset: stair[i, u] = 1 iff u <= i + STAIR_C
F32 = mybir.dt.float32
BF16 = mybir.dt.bfloat16

_NC_CACHE = {}


def _np_bf16():
    import ml_dtypes

    return np.dtype(ml_dtypes.bfloat16)


def _row_tiles():
    out = []
    p0 = 0
    while p0 < H:
        out.append((p0, min(MT, H - p0)))
        p0 += MT
    return out


def _col_tiles(pmax):
    """Exact-width column tiles covering q in [0, pmax]."""
    out = []
    q0 = 0
    while q0 <= pmax:
        nd = min(NTS, pmax - q0 + 1)
        out.append((q0, nd))
        q0 += nd
    return out


def _build_program(rep=1):
    """One SPMD program: conv of IPC images [H, W] with per-image bands.

    rep > 1 wraps the whole body in a hardware loop executing it `rep`
    times - benchmarking only."""
    import contextlib

    nc = bacc.Bacc("TRN2", target_bir_lowering=False, debug=False,
                   num_devices=NCORES)
    x = nc.dram_tensor("x", [IPC, H, W], BF16, kind="ExternalInput")
    bands = nc.dram_tensor("bands", [IPC, 128, 2 * S * BANDW], BF16,
                           kind="ExternalInput")
    y = nc.dram_tensor("y", [IPC, H, W], BF16, kind="ExternalOutput")

    row_tiles = _row_tiles()

    with tile.TileContext(nc) as tc:
        with (
            tc.tile_pool(name="const", bufs=1) as cpool,
            tc.tile_pool(name="xin", bufs=3) as xpool,
            tc.tile_pool(name="out", bufs=3) as opool,
            tc.tile_pool(name="psum", bufs=8, space="PSUM") as ppool,
            tc.For_i(0, rep, 1) if rep > 1 else contextlib.nullcontext(),
        ):
            # Per-image Toeplitz bands (2 row-offset variants), resident
            # for the whole kernel.
            bt = cpool.tile([128, IPC * 2 * S * BANDW], BF16)
            for i in range(IPC):
                nc.sync.dma_start(
                    out=bt[:, i * 2 * S * BANDW:(i + 1) * 2 * S * BANDW],
                    in_=bands[i],
                )
            # Staircase causal mask: stair[i, u] = 1 iff i + STAIR_C - u >= 0.
            stair = cpool.tile([128, STAIR_W], F32)
            nc.gpsimd.memset(stair[:], 1.0)
            nc.gpsimd.affine_select(
                out=stair[:], in_=stair[:],
                compare_op=mybir.AluOpType.is_ge, fill=0.0,
                base=STAIR_C, channel_multiplier=1,
                pattern=[[-1, STAIR_W]],
            )

            for i in range(IPC):
                for (p0, M) in row_tiles:
                    _emit_row_group(nc, tc, xpool, opool, ppool, x, y, bands,
                                    bt, stair, i, [(p0, M)])
    nc.compile()
    return nc


def _prep_x_tile(nc, xpool, x, bands, i, p0, M):
    """Load + causally mask one row-tile's input; returns (xt, variant).

    Tile (k, j) = x[h0 + k, j - 5] with h0 = min(p0-5, H-128): the last
    row-tile loads a full 128 rows ending at the image bottom and uses the
    shifted band variant (no uninitialized partitions anywhere)."""
    pmax = p0 + M - 1
    inw = pmax + 1 + 2 * PW
    h0 = min(p0 - PH, H - 128)
    variant = 1 if h0 != p0 - PH else 0
    hv0 = max(0, h0)
    k0, k1 = hv0 - h0, p0 + M - h0
    xt = xpool.tile([128, inw], BF16, tag="xin")
    # Split the load into <=2 chunks so it lands on 2 DMA queues.
    wmid = (pmax + 1) // 2 if pmax + 1 > 1024 else pmax + 1
    nc.sync.dma_start(out=xt[k0:k1, PW:PW + wmid],
                      in_=x[i, hv0:p0 + M, 0:wmid])
    if wmid < pmax + 1:
        nc.sync.dma_start(out=xt[k0:k1, PW + wmid:PW + pmax + 1],
                          in_=x[i, hv0:p0 + M, wmid:pmax + 1])

    # Causal mask on the diagonal-crossing columns: keep iff
    # (h0 + k) >= (j - 5)  <=>  k - j + (h0 + 5) >= 0.  Only columns
    # j >= h0 + PW can violate it; that region also covers the unloaded
    # right pad (w > pmax >= h there).
    j0 = h0 + PW
    nc.gpsimd.affine_select(
        out=xt[:, j0:], in_=xt[:, j0:],
        compare_op=mybir.AluOpType.is_ge, fill=0.0,
        base=h0 + PW - j0, channel_multiplier=1,
        pattern=[[-1, inw - j0]],
    )
    # Left zero pad (w in [-5, 0)).  Sourced by DMA from the bands
    # tensor's last PW columns, which are structurally all-zero (band
    # entries k-m-off >= R) - a gpsimd memset here raced with the matmul
    # reads on ~1/3 of runs (intermittent q<=5 corruption); DMA-write
    # dependency tracking is solid.  Also kills any k<5 garbage for
    # p0 == 0 (the causal select handles k<5 for j >= 5: h < 0 < w).
    nc.sync.dma_start(out=xt[:, 0:PW],
                      in_=bands[i, :, 2 * S * BANDW - PW:2 * S * BANDW])
    return xt, variant


def _emit_row_group(nc, tc, xpool, opool, ppool, x, y, bands, bt, stair, i,
                    tiles):
    """Emit a group of row-tiles sharing one s-outer matmul loop."""
    work = []  # (p0, M, pmax, col_tiles, xt, variant, pts)
    for (p0, M) in tiles:
        pmax = p0 + M - 1
        col_tiles = _col_tiles(pmax)
        xt, variant = _prep_x_tile(nc, xpool, x, bands, i, p0, M)
        pts = []
        for ct, (q0, nd) in enumerate(col_tiles):
            pt = ppool.tile([M, nd], F32, tag="psum", name=f"pt{p0}_{ct}")
            pts.append(pt)
        work.append((p0, M, pmax, col_tiles, xt, variant, pts))

    for s in range(S):
        for (p0, M, pmax, col_tiles, xt, variant, pts) in work:
            boff = ((i * 2 + variant) * S + s) * BANDW
            band_s = bt[:, boff:boff + M]
            for ct, (q0, nd) in enumerate(col_tiles):
                nc.tensor.matmul(
                    pts[ct][:, :nd],
                    lhsT=band_s,
                    rhs=xt[:, q0 + s:q0 + s + nd],
                    start=(s == 0), stop=(s == S - 1),
                )

    for (p0, M, pmax, col_tiles, xt, variant, pts) in work:
        ot = opool.tile([128, pmax + 1], BF16, tag="out", name=f"ot{p0}")
        for ct, (q0, nd) in enumerate(col_tiles):
            # Causal staircase on tiles crossing p < q: keep iff
            # (p0+m) >= (q0+n) <=> stair[m, n + STAIR_C - (p0-q0)].
            if p0 < q0 + nd - 1:
                u0 = STAIR_C - (p0 - q0)
                nc.vector.tensor_mul(
                    ot[:M, q0:q0 + nd], pts[ct][:M, :nd],
                    stair[:M, u0:u0 + nd],
                )
            else:
                nc.any.tensor_copy(ot[:M, q0:q0 + nd], pts[ct][:M, :nd])
        nc.sync.dma_start(out=y[i, p0:p0 + M, 0:pmax + 1],
                          in_=ot[:M, :pmax + 1])


# Row offset of the shifted band variant: the last row-tile (p0 = 1968,
# M = 80) loads input rows [H-128, H), so k = m + OFF1 + r.
OFF1 = MT * (H // MT) - PH - (H - 128)


def _build_bands(weight):
    """Host-side: per-image banded Toeplitz weights, 2 variants.
    bands[img, k, (v*S+s)*BANDW + m] = w[c(img), k-m-off_v, s] for
    k-m-off_v in [0, R), off_0 = 0, off_1 = OFF1."""
    nimg = N * C
    bands = np.zeros((nimg, 128, 2 * S * BANDW), np.float32)
    m = np.arange(BANDW)
    for v, off in enumerate((0, OFF1)):
        for s in range(S):
            for r in range(R):
                valid = m + r + off < 128
                mv = m[valid]
                for img in range(nimg):
                    c = img % C
                    bands[img, mv + r + off, (v * S + s) * BANDW + mv] = \
                        weight[c, r, s]
    return bands.astype(_np_bf16())


def _bench_inputs(inputs):
    """Full (all-core concatenated) input arrays keyed by DRAM tensor name."""
    x_imgs = np.ascontiguousarray(
        np.asarray(inputs["x"], np.float32).reshape(N * C, H, W)
    ).astype(_np_bf16(), copy=False)
    return {"x": x_imgs, "bands": _build_bands(np.asarray(inputs["weight"]))}


def _get_runner():
    """Jitted SPMD launcher, built once and cached: shard_map over 8 cores
    with donated zero output buffers (mirrors bass2jax.run_bass_via_pjrt,
    but reusable across kernel() calls)."""
    if "runner" in _NC_CACHE:
        return _NC_CACHE["runner"]
    import jax
    import jax.numpy as jnp
    from jax.sharding import Mesh, NamedSharding, PartitionSpec
    from jax.experimental.shard_map import shard_map
    from concourse.bass2jax import (
        _bass_exec_p,
        install_neuronx_cc_hook,
        partition_id_tensor,
    )

    if "nc" not in _NC_CACHE:
        _NC_CACHE["nc"] = _build_program()
    nc = _NC_CACHE["nc"]
    install_neuronx_cc_hook()

    partition_name = (nc.partition_id_tensor.name
                      if nc.partition_id_tensor else None)
    in_names, out_names, out_avals, zero_shapes = [], [], [], []
    for alloc in nc.m.functions[0].allocations:
        if not isinstance(alloc, mybir.MemoryLocationSet):
            continue
        name = alloc.memorylocations[0].name
        if alloc.kind == "ExternalInput":
            if name != partition_name:
                in_names.append(name)
        elif alloc.kind == "ExternalOutput":
            shape = tuple(alloc.tensor_shape)
            dtype = mybir.dt.np(alloc.dtype)
            out_names.append(name)
            out_avals.append(jax.core.ShapedArray(shape, dtype))
            zero_shapes.append((shape, dtype))
    n_params = len(in_names)
    all_in_names = list(in_names) + list(out_names)
    if partition_name is not None:
        all_in_names.append(partition_name)

    def _body(*args):
        operands = list(args)
        if partition_name is not None:
            operands.append(partition_id_tensor())
        outs = _bass_exec_p.bind(
            *operands,
            out_avals=tuple(out_avals),
            in_names=tuple(all_in_names),
            out_names=tuple(out_names),
            lowering_input_output_aliases=(),
            sim_require_finite=True,
            sim_require_nnan=True,
            nc=nc,
        )
        return tuple(outs)

    devices = jax.devices()[:NCORES]
    mesh = Mesh(np.asarray(devices), ("core",))
    sharding = NamedSharding(mesh, PartitionSpec("core"))
    n_outs = len(out_avals)
    f = jax.jit(
        shard_map(_body, mesh=mesh,
                  in_specs=(PartitionSpec("core"),) * (n_params + n_outs),
                  out_specs=(PartitionSpec("core"),) * n_outs,
                  check_rep=False),
        donate_argnums=tuple(range(n_params, n_params + n_outs)),
        keep_unused=True,
    )
    zeros_fns = [
        jax.jit(lambda s=s, d=d: jnp.zeros((NCORES * s[0], *s[1:]), d),
                out_shardings=sharding)
        for (s, d) in zero_shapes
    ]
    runner = (f, in_names, zeros_fns)
    _NC_CACHE["runner"] = runner
    return runner


def kernel(x, weight):
    import jax

    x = np.asarray(x, dtype=np.float32)
    weight = np.asarray(weight, dtype=np.float32)
    assert x.shape == (N, C, H, W) and weight.shape == (C, R, S)

    f, in_names, zeros_fns = _get_runner()
    per_core = _bench_inputs({"x": x, "weight": weight})
    concat_in = [np.ascontiguousarray(per_core[n]) for n in in_names]
    zs = [zf() for zf in zeros_fns]
    out_arrs = f(*concat_in, *zs)
    out = np.asarray(jax.block_until_ready(out_arrs)[0])
    return out.reshape(N, C, H, W).astype(np.float32)
